# revision 1
# baseline (speedup 1.0000x reference)
"""Trainium2 Bass kernel for nn_CausalSelfAttention (BitLinear QKV/O + RoPE + causal attn).

Sharding: 2 heads x 2 batches per core (head-parallel). Each core computes its
heads' q/k/v projections (fp32r matmuls at full PE rate), RoPE, causal
flash-style attention in [k, q] score layout with an unnormalized softmax
(denominator via an appended ones column on V), and a column-sharded output
projection producing a partial [4096, 1024] that the host sums across cores.
"""
import sys

sys.path.insert(0, "/opt/trn_rl_repo")

import numpy as np

GROUP = 128
N_HEADS = 16
EPS = 1e-8
B, T, C = 2, 2048, 1024
HD = 64
N_CORES = 8
HPC = N_HEADS // N_CORES  # 2 heads per core


# ---------------------------------------------------------------- host prep
def _ternary_quantize(w):
    O, I = w.shape
    g = w.reshape(O, I // GROUP, GROUP).astype(np.float32)
    scale = np.maximum(np.mean(np.abs(g), axis=-1, keepdims=True), EPS).astype(
        np.float32
    )
    wn = g / scale
    q = np.where(wn > 0.5, 1.0, np.where(wn < -0.5, -1.0, 0.0)).astype(np.float32)
    return (q * scale).reshape(O, I).astype(np.float32)


def _make_core_inputs(x, wq, wk, wv, wo, rope_cos, rope_sin):
    """Returns list of 8 per-core input dicts (float32, device layouts)."""
    x = np.ascontiguousarray(x.astype(np.float32).reshape(B * T, C))
    wq_q = _ternary_quantize(wq)
    wk_q = _ternary_quantize(wk)
    wv_q = _ternary_quantize(wv)
    wo_q = _ternary_quantize(wo)

    xT = x.T  # [1024 c, 4096 t]
    xt_slab = np.ascontiguousarray(
        xT.reshape(8, 128, 8, 512).transpose(2, 1, 0, 3)
    ).astype(np.float32)  # [s, p, cc, u]

    cosT = rope_cos.astype(np.float32).T  # [32, 2048]
    sinT = rope_sin.astype(np.float32).T
    cos_t = np.tile(cosT, (4, 1)).astype(np.float32)
    sin_t = np.concatenate([-sinT, sinT, -sinT, sinT], axis=0).astype(np.float32)
    tri = (np.arange(128)[None, :] >= np.arange(128)[:, None]).astype(np.float32)
    ident = np.eye(128, dtype=np.float32)

    maps = []
    for core in range(N_CORES):
        r0 = core * HPC * HD
        rows = slice(r0, r0 + HPC * HD)

        def w_lhsT(w_qq):
            wsT = w_qq[rows, :].T  # [1024 in, 128 d]
            return np.ascontiguousarray(
                wsT.reshape(8, 128, 128).transpose(1, 0, 2)
            ).astype(np.float32)

        woc = wo_q[:, rows]  # [1024 o, 128 d]
        maps.append(
            {
                "xt": xt_slab,
                "wqT": w_lhsT(wq_q),
                "wkT": w_lhsT(wk_q),
                "wvT": w_lhsT(wv_q),
                "woTA": np.ascontiguousarray(woc[:, :HD].T),
                "woTB": np.ascontiguousarray(woc[:, HD:].T),
                "cos_t": cos_t,
                "sin_t": sin_t,
                "tri": tri,
                "ident": ident,
                "ones65": np.ones((65, 64), np.float32),
                "vinit": np.ones((128, 64 * 65), np.float32),
            }
        )
    return maps


# ---------------------------------------------------------------- BIR post-pass
def _split_excess_waits(nc, max_waits=1):
    """walrus CoreV3 codegen rejects instructions with >1 sem wait; split the
    excess into preceding NoOps on the same engine."""
    import concourse.mybir as mybir

    for f in nc.m.functions:
        for bb in f.blocks:
            insts = bb.instructions
            i = 0
            while i < len(insts):
                ins = insts[i]
                si = ins.sync_info
                if si is not None and si.on_wait and len(si.on_wait) > max_waits:
                    waits = list(si.on_wait)
                    si.on_wait = waits[:max_waits]
                    rest = waits[max_waits:]
                    new_ops = []
                    for j in range(0, len(rest), max_waits):
                        new_ops.append(
                            mybir.InstNoOp(
                                name=nc.get_next_instruction_name(),
                                sync_info=mybir.SyncInfo(
                                    on_wait=rest[j : j + max_waits], on_update=[]
                                ),
                                bass_nofuse=True,
                                engine=ins.engine,
                            )
                        )
                    insts[i:i] = new_ops
                    i += len(new_ops)
                i += 1


# ---------------------------------------------------------------- device kernel
def _emit(nc, tc, d):
    import concourse.mybir as mybir
    from concourse.bass import ds, ts

    f32 = mybir.dt.float32
    f32r = mybir.dt.float32r
    AF = mybir.ActivationFunctionType
    OP = mybir.AluOpType

    with nc.allow_low_precision(reason="fp32r feeds matmuls; fp32 accum in PSUM"), tc.tile_pool(
        name="const", bufs=1
    ) as cp, tc.tile_pool(name="persist", bufs=1) as pp:
        wq_t = cp.tile([128, 8, 128], f32r)
        nc.sync.dma_start(wq_t[:], d["wqT"])
        wk_t = cp.tile([128, 8, 128], f32r)
        nc.sync.dma_start(wk_t[:], d["wkT"])
        wv_t = cp.tile([128, 8, 128], f32r)
        nc.sync.dma_start(wv_t[:], d["wvT"])
        woC = cp.tile([128, 1024], f32r)
        nc.sync.dma_start(woC[0:64, :], d["woTA"])
        nc.sync.dma_start(woC[64:128, :], d["woTB"])
        cos_sb = cp.tile([128, 2048], f32)
        nc.sync.dma_start(cos_sb[:], d["cos_t"])
        sin_sb = cp.tile([128, 2048], f32)
        nc.sync.dma_start(sin_sb[:], d["sin_t"])
        tri_t = cp.tile([128, 128], f32)
        nc.sync.dma_start(tri_t[:], d["tri"])
        id_t = cp.tile([128, 128], f32)
        nc.sync.dma_start(id_t[:], d["ident"])
        ones65 = cp.tile([65, 64], f32r)
        nc.sync.dma_start(ones65[:], d["ones65"])

        qT = pp.tile([128, 4096], f32r)
        kT = pp.tile([128, 4096], f32r)
        v_sb = pp.tile([128, 64 * 65], f32r)
        y2 = pp.tile([128, 4096], f32r)
        y2B = pp.tile([64, 4096], f32r)
        nc.sync.dma_start(v_sb[:], d["vinit"])  # bakes the ones column of v_aug

        # ---- Phase A: projections (fp32r, N=512) + v transpose to [k, d]
        with tc.tile_pool(name="xt", bufs=2) as xtp, tc.tile_pool(
            name="prps", bufs=3, space="PSUM"
        ) as prps, tc.tile_pool(name="vT", bufs=1) as vtp, tc.tile_pool(
            name="tpps", bufs=2, space="PSUM"
        ) as tpps:
            vT = vtp.tile([128, 4096], f32)
            for s in range(8):
                xt_t = xtp.tile([128, 8, 512], f32r)
                nc.sync.dma_start(xt_t[:], d["xt"][s])
                for w_t, dest in ((wq_t, qT), (wk_t, kT), (wv_t, vT)):
                    ps = prps.tile([128, 512], f32)
                    for j in range(8):
                        nc.tensor.matmul(
                            ps[:],
                            w_t[:, j, :],
                            xt_t[:, j, :],
                            start=(j == 0),
                            stop=(j == 7),
                        )
                    nc.vector.tensor_copy(dest[:, ts(s, 512)], ps[:])

            # ---- Phase A2: RoPE on qT, kT (per batch)
            with tc.tile_pool(name="rope", bufs=2) as rp:
                for tns in (qT, kT):
                    for b in range(2):
                        bcols = ds(b * 2048, 2048)
                        sw = rp.tile([128, 2048], f32r, tag="sw")
                        nc.sync.dma_start(sw[0:32, :], tns[32:64, bcols])
                        nc.sync.dma_start(sw[32:64, :], tns[0:32, bcols])
                        nc.sync.dma_start(sw[64:96, :], tns[96:128, bcols])
                        nc.sync.dma_start(sw[96:128, :], tns[64:96, bcols])
                        tmp = rp.tile([128, 2048], f32, tag="tmp")
                        nc.vector.tensor_tensor(
                            tmp[:], tns[:, bcols], cos_sb[:], OP.mult
                        )
                        nc.vector.tensor_tensor(sw[:], sw[:], sin_sb[:], OP.mult)
                        nc.vector.tensor_tensor(tns[:, bcols], tmp[:], sw[:], OP.add)

            # v transposes into v_sb blocks of 65 (col 64 stays 1.0)
            for h in range(2):
                for b in range(2):
                    for j in range(16):
                        tp = tpps.tile([128, 64], f32)
                        idsl = id_t[64 * h : 64 * h + 64, 64 * h : 64 * h + 64]
                        nc.tensor.transpose(
                            tp[:],
                            vT[64 * h : 64 * h + 64, ds(b * 2048 + j * 128, 128)],
                            idsl,
                        )
                        blk = (h * 2 + b) * 16 + j
                        nc.vector.tensor_copy(v_sb[:, ds(blk * 65, 64)], tp[:])

        # ---- Phase B: attention
        with tc.tile_pool(name="eP", bufs=4) as ep, tc.tile_pool(
            name="rcP", bufs=2
        ) as rcp, tc.tile_pool(name="rbP", bufs=2) as rbp, tc.tile_pool(
            name="sps", bufs=3, space="PSUM"
        ) as sps, tc.tile_pool(name="yps", bufs=2, space="PSUM") as yps, tc.tile_pool(
            name="rbps", bufs=1, space="PSUM"
        ) as rbps:
            for b in range(2):
                for qi in range(4):
                    qcols = ds(b * 2048 + qi * 512, 512)
                    nj = 4 * qi + 4
                    yps_h = [yps.tile([65, 512], f32, name="ypA", tag="ypA"),
                             yps.tile([65, 512], f32, name="ypB", tag="ypB")]
                    for j in range(nj):
                        dlt = j * 128 - qi * 512
                        dlt0 = max(dlt, 0)
                        for h in range(2):
                            sp = sps.tile([128, 512], f32)
                            nc.tensor.matmul(
                                sp[:],
                                kT[
                                    64 * h : 64 * h + 64, ds(b * 2048 + j * 128, 128)
                                ],
                                qT[64 * h : 64 * h + 64, qcols],
                                start=True,
                                stop=True,
                            )
                            E = ep.tile([128, 512], f32r)
                            if dlt < 0:
                                nc.scalar.activation(E[:], sp[:], AF.Exp, scale=0.125)
                            else:
                                nc.scalar.activation(
                                    E[:, ds(dlt, 512 - dlt)],
                                    sp[:, ds(dlt, 512 - dlt)],
                                    AF.Exp,
                                    scale=0.125,
                                )
                                nc.vector.tensor_tensor(
                                    E[:, ds(dlt, 128)],
                                    E[:, ds(dlt, 128)],
                                    tri_t[:],
                                    OP.mult,
                                )
                            blk = (h * 2 + b) * 16 + j
                            nc.tensor.matmul(
                                yps_h[h][:, ds(dlt0, 512 - dlt0)],
                                v_sb[:, ds(blk * 65, 65)],
                                E[:, ds(dlt0, 512 - dlt0)],
                                start=(j == 0),
                                stop=(j == nj - 1),
                                skip_group_check=True,
                            )
                    for h in range(2):
                        yp = yps_h[h]
                        rc = rcp.tile([65, 512], f32r)
                        nc.vector.reciprocal(rc[64:65, :], yp[64:65, :])
                        rbq = rbps.tile([64, 512], f32)
                        nc.tensor.matmul(
                            rbq[:],
                            ones65[64:65, :],
                            rc[64:65, :],
                            start=True,
                            stop=True,
                        )
                        rb = rbp.tile([64, 512], f32)
                        nc.vector.tensor_copy(rb[:], rbq[:])
                        dst = y2[0:64, qcols] if h == 0 else y2B[:, qcols]
                        nc.vector.tensor_tensor(dst, yp[0:64, :], rb[:], OP.mult)
                    nc.sync.dma_start(y2[64:128, qcols], y2B[:, qcols])

        # ---- Phase C: output projection (partial over this core's heads)
        with tc.tile_pool(name="obP", bufs=4) as obp, tc.tile_pool(
            name="ops", bufs=2, space="PSUM"
        ) as ops:
            for tcki in range(32):
                for oc in range(2):
                    op = ops.tile([128, 512], f32)
                    nc.tensor.matmul(
                        op[:],
                        y2[:, ts(tcki, 128)],
                        woC[:, ts(oc, 512)],
                        start=True,
                        stop=True,
                    )
                    ob = obp.tile([128, 512], f32)
                    if oc == 0:
                        nc.vector.tensor_copy(ob[:], op[:])
                    else:
                        nc.scalar.copy(ob[:], op[:])
                    nc.sync.dma_start(
                        d["outp"][ds(tcki * 128, 128), ds(oc * 512, 512)], ob[:]
                    )


_NC_CACHE = {}


def _build():
    if "nc" in _NC_CACHE:
        return _NC_CACHE["nc"]
    import concourse.bass as bass
    import concourse.mybir as mybir
    import concourse.tile as tile

    f32 = mybir.dt.float32
    f32r = mybir.dt.float32r
    nc = bass.Bass("TRN2", target_bir_lowering=False, debug=False, num_devices=1)
    d = {
        "xt": nc.dram_tensor("xt", [8, 128, 8, 512], f32r, kind="ExternalInput").ap(),
        "wqT": nc.dram_tensor("wqT", [128, 8, 128], f32r, kind="ExternalInput").ap(),
        "wkT": nc.dram_tensor("wkT", [128, 8, 128], f32r, kind="ExternalInput").ap(),
        "wvT": nc.dram_tensor("wvT", [128, 8, 128], f32r, kind="ExternalInput").ap(),
        "woTA": nc.dram_tensor("woTA", [64, 1024], f32r, kind="ExternalInput").ap(),
        "woTB": nc.dram_tensor("woTB", [64, 1024], f32r, kind="ExternalInput").ap(),
        "cos_t": nc.dram_tensor("cos_t", [128, 2048], f32, kind="ExternalInput").ap(),
        "sin_t": nc.dram_tensor("sin_t", [128, 2048], f32, kind="ExternalInput").ap(),
        "tri": nc.dram_tensor("tri", [128, 128], f32, kind="ExternalInput").ap(),
        "ident": nc.dram_tensor("ident", [128, 128], f32, kind="ExternalInput").ap(),
        "ones65": nc.dram_tensor("ones65", [65, 64], f32r, kind="ExternalInput").ap(),
        "vinit": nc.dram_tensor("vinit", [128, 64 * 65], f32r, kind="ExternalInput").ap(),
        "outp": nc.dram_tensor(
            "outp", [4096, 1024], f32, kind="ExternalOutput"
        ).ap(),
    }
    with tile.TileContext(nc) as tc:
        _emit(nc, tc, d)
    _split_excess_waits(nc)
    _NC_CACHE["nc"] = nc
    return nc


def kernel(x, wq, wk, wv, wo, rope_cos, rope_sin):
    from concourse import bass_utils

    x, wq, wk, wv, wo, rope_cos, rope_sin = (
        np.asarray(a, dtype=np.float32)
        for a in (x, wq, wk, wv, wo, rope_cos, rope_sin)
    )
    in_maps = _make_core_inputs(x, wq, wk, wv, wo, rope_cos, rope_sin)
    nc = _build()
    res = bass_utils.run_bass_kernel_spmd(nc, in_maps, core_ids=list(range(N_CORES)))
    total = np.zeros((B * T, C), np.float32)
    for i in range(N_CORES):
        total += res.results[i]["outp"]
    return total.reshape(B, T, C).astype(np.float32)

